# revision 5
# baseline (speedup 1.0000x reference)
"""GCN layer on 8 Trainium2 NeuronCores.

out = relu(D^{-1/2} (A+I) D^{-1/2} x W^T + b),  N=8192, D=512, A symmetric binary.

Sharding (1-D graph partition, rank c owns nodes [c*1024, (c+1)*1024)):
  - A+I is symmetric, so the row-block (A+I)[own, :] each core must aggregate
    equals the column slab (A+I)[:, own] transposed: each core is fed the
    natural column slab, which is exactly the [K, N] layout the PE wants.
  - All normalization is folded on the host (deg is a cheap host reduction).
    The device does exactly two matmuls and a scaled relu:
      hT[feat, own] = y^T @ slab      (contract over all 8192 nodes)
      out[own, :]   = relu(d_own^{-1/2} * (hT^T @ W^T) + b)
    No collectives, no cast-DMAs, no on-device degree pass.
  - FP8 path (default): the PE runs the aggregation matmul in DoubleRow fp8
    at 2x bf16 throughput. The slab stays binary {0,1} (exact in e4m3); y is
    split into hi+lo e4m3 halves (y*16 = hi + lo, each quantized) that ride
    the two DoubleRow k-slots against a broadcast slab value, recovering
    ~bf16 precision: sum_i lhsT[p,i,m]*rhs[p,i,n] with rhs[p,0,n]=rhs[p,1,n].
  - Streams: slab chunks on the SP HWDGE queue, y/wt on the ACT HWDGE queue,
    rotating tile-pool buffers so DMA stays ahead of the PE.
"""

import numpy as np

N = 8192
D = 512
NCORES = 8
B = N // NCORES          # 1024 nodes per core
P = 128
KT = N // P              # 64 k-tiles of 128 rows
SCH = 8                  # slab chunks (8 k-tiles each)
SKPC = KT // SCH         # k-tiles per slab chunk
YCH = 4                  # y chunks (16 k-tiles each)
YKPC = KT // YCH
YS = 16.0                # fp8 pre-scale for y (max |16*y| ~ 14.5 < 240)

FP8 = True               # aggregation matmul in DoubleRow fp8 (else bf16)

_cache = {}


def _build(with_bias: bool, ar_chunks: int = 1, reps: int = 1,
           serialize_reps: bool = False, skip_collectives: bool = False,
           num_devices: int = NCORES, mm_n1024: bool = False):
    import concourse.tile as tile
    from concourse import bacc, mybir

    f32 = mybir.dt.float32
    bf16 = mybir.dt.bfloat16
    f8 = mybir.dt.float8e4
    fp8 = FP8

    nc = bacc.Bacc("TRN2", target_bir_lowering=False, debug=False,
                   num_devices=num_devices)

    if fp8:
        slab_d = nc.dram_tensor("slab", [N, B], f8, kind="ExternalInput").ap()
        y_d = nc.dram_tensor("y", [N, 2, D], f8, kind="ExternalInput").ap()
        dvo_d = nc.dram_tensor("dvo", [P, SCH], f32, kind="ExternalInput").ap()
    else:
        slab_d = nc.dram_tensor("slab", [N, B], bf16, kind="ExternalInput").ap()
        y_d = nc.dram_tensor("y", [N, D], bf16, kind="ExternalInput").ap()
    wt_d = nc.dram_tensor("wt", [D, D], bf16, kind="ExternalInput").ap()
    if with_bias:
        bb_d = nc.dram_tensor("bb", [P, D], f32, kind="ExternalInput").ap()
    out_d = nc.dram_tensor("out", [B, D], f32, kind="ExternalOutput").ap()
    out_r = out_d.rearrange("(m p) f -> p m f", p=P)

    dr = mybir.MatmulPerfMode.DoubleRow

    with tile.TileContext(nc) as tc:
        with tc.tile_pool(name="slab", bufs=1) as slab_pool, \
             tc.tile_pool(name="y", bufs=1) as y_pool, \
             tc.tile_pool(name="small", bufs=1) as small, \
             tc.tile_pool(name="osb", bufs=1) as osb_pool, \
             tc.tile_pool(name="psum", bufs=1, space="PSUM") as psum_pool:
          for _rep in range(reps):
            # ---- input streams (HWDGE: slab on SP, y/wt on ACT) ----
            slab_sb = []
            for ch in range(SCH):
                t = slab_pool.tile([P, SKPC, B], f8 if fp8 else bf16,
                                   name=f"slab{ch}", tag=f"sl{ch % 4}")
                src = slab_d[ch * (SKPC * P):(ch + 1) * (SKPC * P), :]
                nc.sync.dma_start(t[:], src.rearrange("(n p) f -> p n f", p=P))
                slab_sb.append(t)
            wt_sb = small.tile([P, D // P, D], bf16, name="wt_sb", tag="wt",
                               bufs=2)
            nc.scalar.dma_start(wt_sb[:],
                                wt_d.rearrange("(kf p) f -> p kf f", p=P))
            if fp8:
                dvo_sb = small.tile([P, SCH], f32, name="dvo_sb", tag="dvo",
                                    bufs=2)
                nc.scalar.dma_start(dvo_sb[:], dvo_d[:])
            if with_bias:
                bb = small.tile([P, D], f32, name="bb_sb", tag="bb", bufs=2)
                nc.scalar.dma_start(bb[:], bb_d[:])
            y_sb = []
            for ch in range(YCH):
                if fp8:
                    t = y_pool.tile([P, YKPC, 2, D], f8, name=f"y{ch}",
                                    tag=f"y{ch % 4}")
                    src = y_d[ch * (YKPC * P):(ch + 1) * (YKPC * P), :, :]
                    nc.scalar.dma_start(
                        t[:], src.rearrange("(n p) i f -> p n i f", p=P))
                else:
                    t = y_pool.tile([P, YKPC, D], bf16, name=f"y{ch}",
                                    tag=f"y{ch % 4}")
                    src = y_d[ch * (YKPC * P):(ch + 1) * (YKPC * P), :]
                    nc.scalar.dma_start(
                        t[:], src.rearrange("(n p) f -> p n f", p=P))
                y_sb.append(t)

            # ---- matmul 1: hT[feat, own] += y_kt^T @ slab_kt ----
            hT_ps = [psum_pool.tile([P, 512], f32, name=f"ps_{j}",
                                    tag=f"ps_{j}") for j in range(8)]
            for kt in range(KT):
                sch, si = divmod(kt, SKPC)
                ych, yi = divmod(kt, YKPC)
                for mf in range(4):
                    if fp8:
                        lhs = y_sb[ych][:, yi, :, mf * P:(mf + 1) * P]
                    else:
                        lhs = y_sb[ych][:, yi, mf * P:(mf + 1) * P]
                    for h in range(2):
                        rhs = slab_sb[sch][:, si, h * 512:(h + 1) * 512]
                        if fp8:
                            rhs = rhs.unsqueeze(1).broadcast_to([P, 2, 512])
                        nc.tensor.matmul(
                            hT_ps[mf * 2 + h], lhsT=lhs, rhs=rhs,
                            start=(kt == 0), stop=(kt == KT - 1),
                            perf_mode=dr if fp8 else None)

            # ---- evacuate hT -> bf16 SBUF [feat_part, 4, own] ----
            hT_sb = small.tile([P, 4, B], bf16, name="hT_sb", tag="hT",
                               bufs=2)
            for h in range(2):
                for mf in range(4):
                    nc.vector.tensor_copy(
                        hT_sb[:, mf, h * 512:(h + 1) * 512],
                        hT_ps[mf * 2 + h][:])

            # ---- matmul 2 + scale/relu: out = relu(dvo * hT^T @ W^T + b) ----
            for m in range(SCH):
                o_ps = psum_pool.tile([P, D], f32, name=f"ops_{m}",
                                      tag=f"ps_{m}")
                for kf in range(4):
                    nc.tensor.matmul(o_ps,
                                     lhsT=hT_sb[:, kf, m * P:(m + 1) * P],
                                     rhs=wt_sb[:, kf, :],
                                     start=(kf == 0), stop=(kf == 3))
                o_sb = osb_pool.tile([P, D], f32, name=f"osb{m}",
                                     tag=f"osb{m % 2}", bufs=2)
                if fp8:
                    if with_bias:
                        nc.vector.tensor_scalar_mul(o_sb[:], o_ps[:],
                                                    dvo_sb[:, m:m + 1])
                        nc.vector.tensor_add(o_sb[:], o_sb[:], bb[:])
                        nc.vector.tensor_scalar_max(o_sb[:], o_sb[:], 0.0)
                    else:
                        nc.vector.tensor_scalar(o_sb[:], o_ps[:],
                                                dvo_sb[:, m:m + 1], 0.0,
                                                mybir.AluOpType.mult,
                                                mybir.AluOpType.max)
                else:
                    if with_bias:
                        nc.vector.tensor_add(o_sb[:], o_ps[:], bb[:])
                        nc.vector.tensor_scalar_max(o_sb[:], o_sb[:], 0.0)
                    else:
                        nc.vector.tensor_scalar_max(o_sb[:], o_ps[:], 0.0)
                nc.sync.dma_start(out_r[:, m, :], o_sb[:])

    nc.compile()
    return nc


def _prep_in_maps(x, A, W, b, with_bias):
    import ml_dtypes
    bf16 = ml_dtypes.bfloat16
    npf8 = ml_dtypes.float8_e4m3

    deg = A.astype(np.float32).sum(axis=1) + 1.0          # A binary, +I
    dv = (1.0 / np.sqrt(deg)).astype(np.float32)
    wt = np.ascontiguousarray(W.astype(np.float32).T).astype(bf16)
    if FP8:
        ys = (YS * dv[:, None] * x.astype(np.float32))
        y_hi = ys.astype(npf8)
        y_lo = (ys - y_hi.astype(np.float32)).astype(npf8)
        y = np.ascontiguousarray(np.stack([y_hi, y_lo], axis=1))  # [N, 2, D]
    else:
        y = (dv[:, None] * x.astype(np.float32)).astype(bf16)
    in_maps = []
    for c in range(NCORES):
        own = slice(c * B, (c + 1) * B)
        sl = np.array(A[:, own], dtype=np.float32)
        sl[np.arange(c * B, (c + 1) * B), np.arange(B)] += 1.0  # fold +I
        if FP8:
            m = {"slab": sl.astype(npf8), "y": y, "wt": wt,
                 "dvo": np.ascontiguousarray(
                     (dv[own] / YS).reshape(SCH, P).T)}
        else:
            sl *= dv[own][None, :]                        # fold d_own^{-1/2}
            m = {"slab": sl.astype(bf16), "y": y, "wt": wt}
        if with_bias:
            m["bb"] = np.ascontiguousarray(
                np.broadcast_to(b.astype(np.float32), (P, D)))
        in_maps.append(m)
    return in_maps


def get_compiled(with_bias, ar_chunks=1, reps=1, serialize_reps=False,
                 skip_collectives=False, num_devices=NCORES, mm_n1024=False):
    key = (FP8, with_bias, ar_chunks, reps, serialize_reps, skip_collectives,
           num_devices, mm_n1024)
    if key not in _cache:
        _cache[key] = _build(with_bias, ar_chunks, reps, serialize_reps,
                             skip_collectives, num_devices, mm_n1024)
    return _cache[key]


def kernel(x, A, W, b):
    from concourse import bass_utils

    with_bias = bool(np.any(b))
    nc = get_compiled(with_bias)
    in_maps = _prep_in_maps(x, A, W, b, with_bias)
    try:
        res = bass_utils.run_bass_kernel_spmd(nc, in_maps,
                                              core_ids=list(range(NCORES)))
    except Exception:
        # the shared terminal occasionally wedges (NRT_EXEC_UNIT_UNRECOVERABLE
        # from a prior session); it auto-resets after ~1 min
        import time
        time.sleep(75)
        res = bass_utils.run_bass_kernel_spmd(nc, in_maps,
                                              core_ids=list(range(NCORES)))
    out = np.concatenate([res.results[c]["out"] for c in range(NCORES)], axis=0)
    return out.astype(np.float32)


# revision 23
# speedup vs baseline: 1.4697x; 1.4697x over previous
"""GCN layer on 8 Trainium2 NeuronCores.

out = relu(D^{-1/2} (A+I) D^{-1/2} x W^T + b),  N=8192, D=512, A symmetric binary.

Sharding (1-D graph partition, rank c owns nodes [c*1024, (c+1)*1024)):
  - A+I is symmetric, so the row-block (A+I)[own, :] each core must aggregate
    equals the column slab (A+I)[:, own] transposed: each core is fed the
    natural column slab, which is exactly the [K, N] layout the PE wants.
  - All normalization is folded on the host (deg is a cheap host reduction):
      slab' = (A+I)[:, own] * d_own^{-1/2}[col]
      y     = d^{-1/2}[:, None] * x               (replicated)
    so the device does exactly two matmuls and a relu:
      hT[feat, own] = y^T @ slab'   (contract over all 8192 nodes)
      out[own, :]   = relu(hT^T @ W^T + b)
    No collectives, no cast-DMAs, no on-device degree pass.
  - The aggregation matmul runs bf16(y) x fp8e4(slab'): binary adjacency
    scaled by d^{-1/2} fits fp8 within bf16-equivalent accuracy while halving
    slab HBM traffic (PE upconverts both operands; rate is bf16's).
  - Streams: slab chunks on the SP HWDGE queue, y/wt on the ACT HWDGE queue,
    out rows on the SWDGE (gpsimd) queue so stores never block input FIFOs.
"""

import numpy as np

N = 8192
D = 512
NCORES = 8
B = N // NCORES          # 1024 nodes per core
P = 128
KT = N // P              # 64 k-tiles of 128 rows
SCH = 8                  # slab chunks (8 k-tiles each)
SKPC = KT // SCH         # k-tiles per chunk
YCH = 4                  # y chunks (16 k-tiles each)
YKPC = KT // YCH

# variant knobs (A/B-able; defaults = current best)
SLAB_FP8 = True          # slab in fp8e4 (mixed-dtype matmul vs bf16 slab)
OUT_SWDGE = True         # out DMA on gpsimd queue (vs sync HWDGE)
QUEUE_BAL = False        # split slab/y across both HWDGE queues evenly
NCHUNKS = (16, 8)        # (slab chunks, y chunks) per rep
PACK_KT = 24             # k-tiles aggregated as fp8 DoubleRow pairs (0..64,
                         # multiple of KT//NCHUNKS[1]); trades accuracy
                         # (e4m3 y on those rows) for 4 MMs saved per pair
YSC = 16.0               # y pre-scale (exact bf16 shift; e4m3 range fit)

_cache = {}


def _build(with_bias: bool, ar_chunks: int = 1, reps: int = 1,
           serialize_reps: bool = False, skip_collectives: bool = False,
           num_devices: int = NCORES, mm_n1024: bool = False):
    import concourse.tile as tile
    from concourse import bacc, mybir

    f32 = mybir.dt.float32
    bf16 = mybir.dt.bfloat16
    sdt = mybir.dt.float8e4 if SLAB_FP8 else bf16

    nc = bacc.Bacc("TRN2", target_bir_lowering=False, debug=False,
                   num_devices=num_devices)

    f8 = mybir.dt.float8e4
    dr = mybir.MatmulPerfMode.DoubleRow
    assert PACK_KT == 0 or SLAB_FP8, "DoubleRow packing needs the fp8 slab"

    slab_d = nc.dram_tensor("slab", [N, B], sdt, kind="ExternalInput").ap()
    y_d = nc.dram_tensor("y", [N, D], bf16, kind="ExternalInput").ap()
    if PACK_KT:
        y8_d = nc.dram_tensor("y8", [PACK_KT * P, D], f8,
                              kind="ExternalInput").ap()
    wt_d = nc.dram_tensor("wt", [D, D], bf16, kind="ExternalInput").ap()
    if SLAB_FP8:
        # fp8 slab stays binary-exact; d_own^{-1/2} applied on evacuation
        dvo_d = nc.dram_tensor("dvo", [P, SCH], f32, kind="ExternalInput").ap()
    if with_bias:
        bb_d = nc.dram_tensor("bb", [P, D], f32, kind="ExternalInput").ap()
    out_d = nc.dram_tensor("out", [B, D], f32, kind="ExternalOutput").ap()
    out_r = out_d.rearrange("(m p) f -> p m f", p=P)

    with tile.TileContext(nc) as tc:
        with tc.tile_pool(name="slab", bufs=1) as slab_pool, \
             tc.tile_pool(name="y", bufs=1) as y_pool, \
             tc.tile_pool(name="small", bufs=1) as small, \
             tc.tile_pool(name="osb", bufs=1) as osb_pool, \
             tc.tile_pool(name="psum", bufs=1, space="PSUM") as psum_pool:
          sch_n, ych_n = NCHUNKS
          skpc, ykpc = KT // sch_n, KT // ych_n
          sslots = 4 if sch_n >= 8 else 2
          yslots = 4 if ych_n >= 4 else 2
          for _rep in range(reps):
            # ---- input streams ----
            slab_sb = []
            for ch in range(sch_n):
                t = slab_pool.tile([P, skpc, B], sdt, name=f"slab{ch}",
                                   tag=f"sl{ch % sslots}")
                src = slab_d[ch * (skpc * P):(ch + 1) * (skpc * P), :]
                eng = nc.scalar if (QUEUE_BAL and ch % 2) else nc.sync
                eng.dma_start(t[:], src.rearrange("(n p) f -> p n f", p=P))
                slab_sb.append(t)
            wt_sb = small.tile([P, D // P, D], bf16, name="wt_sb", tag="wt",
                               bufs=2)
            nc.scalar.dma_start(wt_sb[:],
                                wt_d.rearrange("(kf p) f -> p kf f", p=P))
            if SLAB_FP8:
                dvo_sb = small.tile([P, SCH], f32, name="dvo_sb", tag="dvo",
                                    bufs=2)
                nc.scalar.dma_start(dvo_sb[:], dvo_d[:])
            if with_bias:
                bb = small.tile([P, D], f32, name="bb_sb", tag="bb", bufs=2)
                nc.scalar.dma_start(bb[:], bb_d[:])
            pack_ch = PACK_KT // ykpc        # y chunks covered by packed rows
            assert PACK_KT % ykpc == 0 and PACK_KT % 2 == 0 and skpc % 2 == 0
            y_sb = []
            for ch in range(ych_n):
                if ch < pack_ch:
                    y_sb.append(None)        # packed rows use y8 instead
                    continue
                t = y_pool.tile([P, ykpc, D], bf16, name=f"y{ch}",
                                tag=f"y{ch % yslots}")
                src = y_d[ch * (ykpc * P):(ch + 1) * (ykpc * P), :]
                eng = nc.sync if (QUEUE_BAL and ch % 2) else nc.scalar
                eng.dma_start(t[:], src.rearrange("(n p) f -> p n f", p=P))
                y_sb.append(t)
            y8_sb = []
            for ch in range(pack_ch):
                t = y_pool.tile([P, ykpc, D], f8, name=f"y8{ch}",
                                tag=f"y8{ch % 2}")
                src = y8_d[ch * (ykpc * P):(ch + 1) * (ykpc * P), :]
                nc.scalar.dma_start(t[:],
                                    src.rearrange("(n p) f -> p n f", p=P))
                y8_sb.append(t)

            # ---- matmul 1: hT[feat, own] += y_kt^T @ slab_kt ----
            hT_ps = [psum_pool.tile([P, 512], f32, name=f"ps_{j}",
                                    tag=f"ps_{j}") for j in range(8)]
            # fp8 DoubleRow pairs over the packed prefix: 1 MM contracts 2
            # k-tiles (chunk tiles already hold adjacent k-tiles as the
            # [Ki, 2, dim] pair layout)
            for pt in range(PACK_KT // 2):
                kt0 = 2 * pt
                sch, si = divmod(kt0, skpc)
                ych8, yi8 = divmod(kt0, ykpc)
                for mf in range(4):
                    lhs = y8_sb[ych8][:, yi8:yi8 + 2, mf * P:(mf + 1) * P]
                    for h in range(2):
                        nc.tensor.matmul(
                            hT_ps[mf * 2 + h], lhsT=lhs,
                            rhs=slab_sb[sch][:, si:si + 2,
                                             h * 512:(h + 1) * 512],
                            start=(pt == 0), stop=(PACK_KT == KT
                                                   and pt == PACK_KT // 2 - 1),
                            perf_mode=dr)
            for kt in range(PACK_KT, KT):
                sch, si = divmod(kt, skpc)
                ych, yi = divmod(kt, ykpc)
                for mf in range(4):
                    lhs = y_sb[ych][:, yi, mf * P:(mf + 1) * P]
                    for h in range(2):
                        nc.tensor.matmul(
                            hT_ps[mf * 2 + h], lhsT=lhs,
                            rhs=slab_sb[sch][:, si, h * 512:(h + 1) * 512],
                            start=(kt == PACK_KT and PACK_KT == 0),
                            stop=(kt == KT - 1))

            # ---- evacuate hT -> bf16 SBUF [feat_part, 4, own] ----
            hT_sb = small.tile([P, 4, B], bf16, name="hT_sb", tag="hT",
                               bufs=2)
            for h in range(2):
                for mf in range(4):
                    nc.vector.tensor_copy(
                        hT_sb[:, mf, h * 512:(h + 1) * 512],
                        hT_ps[mf * 2 + h][:])

            # ---- matmul 2 + relu: out = relu(hT^T @ W^T + b) ----
            for m in range(SCH):
                o_ps = psum_pool.tile([P, D], f32, name=f"ops_{m}",
                                      tag=f"ps_{m}")
                for kf in range(4):
                    nc.tensor.matmul(o_ps,
                                     lhsT=hT_sb[:, kf, m * P:(m + 1) * P],
                                     rhs=wt_sb[:, kf, :],
                                     start=(kf == 0), stop=(kf == 3))
                o_sb = osb_pool.tile([P, D], f32, name=f"osb{m}",
                                     tag=f"osb{m % 2}", bufs=2)
                if SLAB_FP8:
                    if with_bias:
                        nc.vector.tensor_scalar_mul(o_sb[:], o_ps[:],
                                                    dvo_sb[:, m:m + 1])
                        nc.vector.tensor_add(o_sb[:], o_sb[:], bb[:])
                        nc.vector.tensor_scalar_max(o_sb[:], o_sb[:], 0.0)
                    else:
                        nc.vector.tensor_scalar(o_sb[:], o_ps[:],
                                                dvo_sb[:, m:m + 1], 0.0,
                                                mybir.AluOpType.mult,
                                                mybir.AluOpType.max)
                elif with_bias:
                    nc.vector.tensor_add(o_sb[:], o_ps[:], bb[:])
                    nc.vector.tensor_scalar_max(o_sb[:], o_sb[:], 0.0)
                else:
                    nc.vector.tensor_scalar_max(o_sb[:], o_ps[:], 0.0)
                oeng = nc.gpsimd if OUT_SWDGE else nc.sync
                oeng.dma_start(out_r[:, m, :], o_sb[:])

    nc.compile()
    return nc


def _prep_in_maps(x, A, W, b, with_bias):
    import ml_dtypes
    bf16 = ml_dtypes.bfloat16
    sdt = ml_dtypes.float8_e4m3 if SLAB_FP8 else bf16

    deg = A.astype(np.float32).sum(axis=1) + 1.0          # A binary, +I
    dv = (1.0 / np.sqrt(deg)).astype(np.float32)
    ysc = YSC if SLAB_FP8 else 1.0        # exact bf16 shift; undone via dvo
    ys = ysc * dv[:, None] * x.astype(np.float32)
    y = ys.astype(bf16)
    y8 = ys[:PACK_KT * P].astype(ml_dtypes.float8_e4m3) if PACK_KT else None
    wt = np.ascontiguousarray(W.astype(np.float32).T).astype(bf16)
    in_maps = []
    for c in range(NCORES):
        own = slice(c * B, (c + 1) * B)
        sl = np.array(A[:, own], dtype=np.float32)
        sl[np.arange(c * B, (c + 1) * B), np.arange(B)] += 1.0  # fold +I
        if SLAB_FP8:
            # keep the slab binary (exact in e4m3); scale rows on evacuation
            m = {"slab": sl.astype(sdt), "y": y, "wt": wt,
                 "dvo": np.ascontiguousarray(
                     (dv[own] / ysc).reshape(SCH, P).T)}
            if PACK_KT:
                m["y8"] = y8
        else:
            sl *= dv[own][None, :]                        # fold d_own^{-1/2}
            m = {"slab": sl.astype(sdt), "y": y, "wt": wt}
        if with_bias:
            m["bb"] = np.ascontiguousarray(
                np.broadcast_to(b.astype(np.float32), (P, D)))
        in_maps.append(m)
    return in_maps


def get_compiled(with_bias, ar_chunks=1, reps=1, serialize_reps=False,
                 skip_collectives=False, num_devices=NCORES, mm_n1024=False):
    key = (SLAB_FP8, OUT_SWDGE, QUEUE_BAL, NCHUNKS, PACK_KT, with_bias,
           ar_chunks, reps, serialize_reps, skip_collectives, num_devices,
           mm_n1024)
    if key not in _cache:
        _cache[key] = _build(with_bias, ar_chunks, reps, serialize_reps,
                             skip_collectives, num_devices, mm_n1024)
    return _cache[key]


def kernel(x, A, W, b):
    from concourse import bass_utils

    with_bias = bool(np.any(b))
    nc = get_compiled(with_bias)
    in_maps = _prep_in_maps(x, A, W, b, with_bias)
    try:
        res = bass_utils.run_bass_kernel_spmd(nc, in_maps,
                                              core_ids=list(range(NCORES)))
    except Exception:
        # the shared terminal occasionally wedges (NRT_EXEC_UNIT_UNRECOVERABLE
        # from a prior session); it auto-resets after ~1 min
        import time
        time.sleep(75)
        res = bass_utils.run_bass_kernel_spmd(nc, in_maps,
                                              core_ids=list(range(NCORES)))
    out = np.concatenate([res.results[c]["out"] for c in range(NCORES)], axis=0)
    return out.astype(np.float32)


# revision 25
# speedup vs baseline: 1.4894x; 1.0134x over previous
"""GCN layer on 8 Trainium2 NeuronCores.

out = relu(D^{-1/2} (A+I) D^{-1/2} x W^T + b),  N=8192, D=512, A symmetric binary.

Sharding (1-D graph partition, rank c owns nodes [c*1024, (c+1)*1024)):
  - A+I is symmetric, so the row-block (A+I)[own, :] each core must aggregate
    equals the column slab (A+I)[:, own] transposed: each core is fed the
    natural column slab, which is exactly the [K, N] layout the PE wants.
  - All normalization is folded on the host (deg is a cheap host reduction):
      slab' = (A+I)[:, own] * d_own^{-1/2}[col]
      y     = d^{-1/2}[:, None] * x               (replicated)
    so the device does exactly two matmuls and a relu:
      hT[feat, own] = y^T @ slab'   (contract over all 8192 nodes)
      out[own, :]   = relu(hT^T @ W^T + b)
    No collectives, no cast-DMAs, no on-device degree pass.
  - The aggregation matmul runs bf16(y) x fp8e4(slab'): binary adjacency
    scaled by d^{-1/2} fits fp8 within bf16-equivalent accuracy while halving
    slab HBM traffic (PE upconverts both operands; rate is bf16's).
  - Streams: slab chunks on the SP HWDGE queue, y/wt on the ACT HWDGE queue,
    out rows on the SWDGE (gpsimd) queue so stores never block input FIFOs.
"""

import numpy as np

N = 8192
D = 512
NCORES = 8
B = N // NCORES          # 1024 nodes per core
P = 128
KT = N // P              # 64 k-tiles of 128 rows
SCH = 8                  # slab chunks (8 k-tiles each)
SKPC = KT // SCH         # k-tiles per chunk
YCH = 4                  # y chunks (16 k-tiles each)
YKPC = KT // YCH

# variant knobs (A/B-able; defaults = current best)
SLAB_FP8 = True          # slab in fp8e4 (mixed-dtype matmul vs bf16 slab)
OUT_SWDGE = True         # out DMA on gpsimd queue (vs sync HWDGE)
QUEUE_BAL = False        # split slab/y across both HWDGE queues evenly
NCHUNKS = (16, 8)        # (slab chunks, y chunks) per rep
PACK_KT = 24             # k-tiles aggregated as fp8 DoubleRow pairs (0..64,
                         # multiple of KT//NCHUNKS[1]); trades accuracy
                         # (e4m3 y on those rows) for 4 MMs saved per pair
YSC = 16.0               # y pre-scale (exact bf16 shift; e4m3 range fit)

_cache = {}


def _build(with_bias: bool, ar_chunks: int = 1, reps: int = 1,
           serialize_reps: bool = False, skip_collectives: bool = False,
           num_devices: int = NCORES, mm_n1024: bool = False):
    import concourse.tile as tile
    from concourse import bacc, mybir

    f32 = mybir.dt.float32
    bf16 = mybir.dt.bfloat16
    sdt = mybir.dt.float8e4 if SLAB_FP8 else bf16

    nc = bacc.Bacc("TRN2", target_bir_lowering=False, debug=False,
                   num_devices=num_devices)

    f8 = mybir.dt.float8e4
    dr = mybir.MatmulPerfMode.DoubleRow
    assert PACK_KT == 0 or SLAB_FP8, "DoubleRow packing needs the fp8 slab"

    slab_d = nc.dram_tensor("slab", [N, B], sdt, kind="ExternalInput").ap()
    y_d = nc.dram_tensor("y", [N, D], bf16, kind="ExternalInput").ap()
    if PACK_KT:
        y8_d = nc.dram_tensor("y8", [PACK_KT * P, D], f8,
                              kind="ExternalInput").ap()
    wt_d = nc.dram_tensor("wt", [D, D], bf16, kind="ExternalInput").ap()
    if SLAB_FP8:
        # fp8 slab stays binary-exact; d_own^{-1/2} applied on evacuation
        dvo_d = nc.dram_tensor("dvo", [P, SCH], f32, kind="ExternalInput").ap()
    if with_bias:
        bb_d = nc.dram_tensor("bb", [P, D], f32, kind="ExternalInput").ap()
    out_d = nc.dram_tensor("out", [B, D], f32, kind="ExternalOutput").ap()
    out_r = out_d.rearrange("(m p) f -> p m f", p=P)

    with tile.TileContext(nc) as tc:
        with tc.tile_pool(name="slab", bufs=1) as slab_pool, \
             tc.tile_pool(name="y", bufs=1) as y_pool, \
             tc.tile_pool(name="small", bufs=1) as small, \
             tc.tile_pool(name="osb", bufs=1) as osb_pool, \
             tc.tile_pool(name="psum", bufs=1, space="PSUM") as psum_pool:
          sch_n, ych_n = NCHUNKS
          skpc, ykpc = KT // sch_n, KT // ych_n
          sslots = 4 if sch_n >= 8 else 2
          yslots = 4 if ych_n >= 4 else 2
          for _rep in range(reps):
            # ---- input streams ----
            slab_sb = []
            for ch in range(sch_n):
                t = slab_pool.tile([P, skpc, B], sdt, name=f"slab{ch}",
                                   tag=f"sl{ch % sslots}")
                src = slab_d[ch * (skpc * P):(ch + 1) * (skpc * P), :]
                eng = nc.scalar if (QUEUE_BAL and ch % 2) else nc.sync
                eng.dma_start(t[:], src.rearrange("(n p) f -> p n f", p=P))
                slab_sb.append(t)
            wt_sb = small.tile([P, D // P, D], bf16, name="wt_sb", tag="wt",
                               bufs=2)
            nc.scalar.dma_start(wt_sb[:],
                                wt_d.rearrange("(kf p) f -> p kf f", p=P))
            if SLAB_FP8:
                dvo_sb = small.tile([P, SCH], f32, name="dvo_sb", tag="dvo",
                                    bufs=2)
                nc.scalar.dma_start(dvo_sb[:], dvo_d[:])
            if with_bias:
                bb = small.tile([P, D], f32, name="bb_sb", tag="bb", bufs=2)
                nc.scalar.dma_start(bb[:], bb_d[:])
            pack_ch = PACK_KT // ykpc        # y chunks covered by packed rows
            assert PACK_KT % ykpc == 0 and PACK_KT % 2 == 0 and skpc % 2 == 0
            y_sb = []
            for ch in range(ych_n):
                if ch < pack_ch:
                    y_sb.append(None)        # packed rows use y8 instead
                    continue
                t = y_pool.tile([P, ykpc, D], bf16, name=f"y{ch}",
                                tag=f"y{ch % yslots}")
                src = y_d[ch * (ykpc * P):(ch + 1) * (ykpc * P), :]
                eng = nc.sync if (QUEUE_BAL and ch % 2) else nc.scalar
                eng.dma_start(t[:], src.rearrange("(n p) f -> p n f", p=P))
                y_sb.append(t)
            y8_sb = []
            for ch in range(pack_ch):
                t = y_pool.tile([P, ykpc, D], f8, name=f"y8{ch}",
                                tag=f"y8{ch % 2}")
                src = y8_d[ch * (ykpc * P):(ch + 1) * (ykpc * P), :]
                nc.scalar.dma_start(t[:],
                                    src.rearrange("(n p) f -> p n f", p=P))
                y8_sb.append(t)

            # ---- matmul 1: hT[feat, own] += y_kt^T @ slab_kt ----
            hT_ps = [psum_pool.tile([P, 512], f32, name=f"ps_{j}",
                                    tag=f"ps_{j}") for j in range(8)]
            # fp8 DoubleRow pairs over the packed prefix: 1 MM contracts 2
            # k-tiles (chunk tiles already hold adjacent k-tiles as the
            # [Ki, 2, dim] pair layout)
            for pt in range(PACK_KT // 2):
                kt0 = 2 * pt
                sch, si = divmod(kt0, skpc)
                ych8, yi8 = divmod(kt0, ykpc)
                for mf in range(4):
                    lhs = y8_sb[ych8][:, yi8:yi8 + 2, mf * P:(mf + 1) * P]
                    for h in range(2):
                        nc.tensor.matmul(
                            hT_ps[mf * 2 + h], lhsT=lhs,
                            rhs=slab_sb[sch][:, si:si + 2,
                                             h * 512:(h + 1) * 512],
                            start=(pt == 0), stop=(PACK_KT == KT
                                                   and pt == PACK_KT // 2 - 1),
                            perf_mode=dr)
            for kt in range(PACK_KT, KT):
                sch, si = divmod(kt, skpc)
                ych, yi = divmod(kt, ykpc)
                for mf in range(4):
                    lhs = y_sb[ych][:, yi, mf * P:(mf + 1) * P]
                    for h in range(2):
                        nc.tensor.matmul(
                            hT_ps[mf * 2 + h], lhsT=lhs,
                            rhs=slab_sb[sch][:, si, h * 512:(h + 1) * 512],
                            start=(kt == PACK_KT and PACK_KT == 0),
                            stop=(kt == KT - 1))

            # ---- evacuate hT -> bf16 SBUF [feat_part, 4, own] ----
            hT_sb = small.tile([P, 4, B], bf16, name="hT_sb", tag="hT",
                               bufs=2)
            for h in range(2):
                for mf in range(4):
                    # split PSUM drain across ScalarE/VectorE (parallel on
                    # different banks)
                    dst = hT_sb[:, mf, h * 512:(h + 1) * 512]
                    src = hT_ps[mf * 2 + h][:]
                    if h == 0:
                        nc.scalar.copy(dst, src)
                    else:
                        nc.vector.tensor_copy(dst, src)

            # ---- matmul 2 + relu: out = relu(hT^T @ W^T + b) ----
            for m in range(SCH):
                o_ps = psum_pool.tile([P, D], f32, name=f"ops_{m}",
                                      tag=f"ps_{m}")
                for kf in range(4):
                    nc.tensor.matmul(o_ps,
                                     lhsT=hT_sb[:, kf, m * P:(m + 1) * P],
                                     rhs=wt_sb[:, kf, :],
                                     start=(kf == 0), stop=(kf == 3))
                o_sb = osb_pool.tile([P, D], f32, name=f"osb{m}",
                                     tag=f"osb{m % 2}", bufs=2)
                if SLAB_FP8:
                    if with_bias:
                        nc.vector.tensor_scalar_mul(o_sb[:], o_ps[:],
                                                    dvo_sb[:, m:m + 1])
                        nc.vector.tensor_add(o_sb[:], o_sb[:], bb[:])
                        nc.vector.tensor_scalar_max(o_sb[:], o_sb[:], 0.0)
                    elif m % 2 == 0:
                        # relu(o_ps * dvo) in one ScalarE op; odd banks on DVE
                        nc.scalar.activation(
                            o_sb[:], o_ps[:],
                            mybir.ActivationFunctionType.Relu,
                            scale=dvo_sb[:, m:m + 1])
                    else:
                        nc.vector.tensor_scalar(o_sb[:], o_ps[:],
                                                dvo_sb[:, m:m + 1], 0.0,
                                                mybir.AluOpType.mult,
                                                mybir.AluOpType.max)
                elif with_bias:
                    nc.vector.tensor_add(o_sb[:], o_ps[:], bb[:])
                    nc.vector.tensor_scalar_max(o_sb[:], o_sb[:], 0.0)
                else:
                    nc.vector.tensor_scalar_max(o_sb[:], o_ps[:], 0.0)
                oeng = nc.gpsimd if OUT_SWDGE else nc.sync
                oeng.dma_start(out_r[:, m, :], o_sb[:])

    nc.compile()
    return nc


def _prep_in_maps(x, A, W, b, with_bias):
    import ml_dtypes
    bf16 = ml_dtypes.bfloat16
    sdt = ml_dtypes.float8_e4m3 if SLAB_FP8 else bf16

    deg = A.astype(np.float32).sum(axis=1) + 1.0          # A binary, +I
    dv = (1.0 / np.sqrt(deg)).astype(np.float32)
    ysc = YSC if SLAB_FP8 else 1.0        # exact bf16 shift; undone via dvo
    ys = ysc * dv[:, None] * x.astype(np.float32)
    y = ys.astype(bf16)
    y8 = ys[:PACK_KT * P].astype(ml_dtypes.float8_e4m3) if PACK_KT else None
    wt = np.ascontiguousarray(W.astype(np.float32).T).astype(bf16)
    in_maps = []
    for c in range(NCORES):
        own = slice(c * B, (c + 1) * B)
        sl = np.array(A[:, own], dtype=np.float32)
        sl[np.arange(c * B, (c + 1) * B), np.arange(B)] += 1.0  # fold +I
        if SLAB_FP8:
            # keep the slab binary (exact in e4m3); scale rows on evacuation
            m = {"slab": sl.astype(sdt), "y": y, "wt": wt,
                 "dvo": np.ascontiguousarray(
                     (dv[own] / ysc).reshape(SCH, P).T)}
            if PACK_KT:
                m["y8"] = y8
        else:
            sl *= dv[own][None, :]                        # fold d_own^{-1/2}
            m = {"slab": sl.astype(sdt), "y": y, "wt": wt}
        if with_bias:
            m["bb"] = np.ascontiguousarray(
                np.broadcast_to(b.astype(np.float32), (P, D)))
        in_maps.append(m)
    return in_maps


def get_compiled(with_bias, ar_chunks=1, reps=1, serialize_reps=False,
                 skip_collectives=False, num_devices=NCORES, mm_n1024=False):
    key = (SLAB_FP8, OUT_SWDGE, QUEUE_BAL, NCHUNKS, PACK_KT, with_bias,
           ar_chunks, reps, serialize_reps, skip_collectives, num_devices,
           mm_n1024)
    if key not in _cache:
        _cache[key] = _build(with_bias, ar_chunks, reps, serialize_reps,
                             skip_collectives, num_devices, mm_n1024)
    return _cache[key]


def kernel(x, A, W, b):
    from concourse import bass_utils

    with_bias = bool(np.any(b))
    nc = get_compiled(with_bias)
    in_maps = _prep_in_maps(x, A, W, b, with_bias)
    try:
        res = bass_utils.run_bass_kernel_spmd(nc, in_maps,
                                              core_ids=list(range(NCORES)))
    except Exception:
        # the shared terminal occasionally wedges (NRT_EXEC_UNIT_UNRECOVERABLE
        # from a prior session); it auto-resets after ~1 min
        import time
        time.sleep(75)
        res = bass_utils.run_bass_kernel_spmd(nc, in_maps,
                                              core_ids=list(range(NCORES)))
    out = np.concatenate([res.results[c]["out"] for c in range(NCORES)], axis=0)
    return out.astype(np.float32)
